# revision 7
# baseline (speedup 1.0000x reference)
"""Trainium2 Bass kernel for ConvMultiStepAttention.

Math (per batch element b):
    preatt = W @ x + b                      # [C,T], x = input_from_dec[b,:,:,0]
    target = (base + preatt) * sqrt(0.5)    # [C,T]
    scores = target.T @ top                 # [T,S]
    attn   = softmax(scores, axis=1)        # [T,S]   (output 2)
    ctx    = attn @ combine.T               # [T,C] -> stored as [C,T] (output 1)

Sharding: pure data parallel, one batch element per NeuronCore (B=8 = n_cores).

Precision: all matmuls run in fp16 (1 cycle/row on PE vs 4 for fp32) with fp32
PSUM accumulation; softmax stats (max/sum) in fp32.  sqrt(0.5) is folded into
W, b and base on the host.  Measured end-to-end absmax error vs the fp32
reference is ~1.1e-2 of output scale.

attn^T (needed as the ctx-matmul moving operand with the contraction dim on
partitions) is produced by reading the already-written attn fp16 DRAM output
back through the DMA xbar transpose in [1024, 128] blocks.  The xbar dispatch
cost is ~1.2us per *instruction* regardless of size, so few big jobs beat many
128x128 ones.  combine^T is pre-transposed on the host.
"""

import numpy as np

_B, _C, _T, _S = 8, 512, 2048, 2048
_SW = np.float32(0.5**0.5)
_P = 128
_NB = 512  # one PSUM bank in fp32 elements; also max fp32 matmul free dim

_cache: dict = {}


def _build(C: int, T: int, S: int):
    """Build + compile the single-core SPMD program. Returns the Bass object."""
    from contextlib import ExitStack

    import concourse.bacc as bacc
    import concourse.tile as tile
    from concourse import mybir
    from concourse.tile_rust import add_dep_helper

    f16 = mybir.dt.float16
    f32 = mybir.dt.float32
    P, NB = _P, _NB
    KC = C // P  # channel k-subtiles            (4)
    KS = S // P  # s k-subtiles for ctx matmul   (16)
    NT = T // P  # t row-tiles                   (16)
    NTC = T // NB  # t chunks of 512             (4)
    TH = T // 2  # t-half size                   (1024)
    HT = NT // 2  # t-tiles per half             (8)

    nc = bacc.Bacc(
        "TRN2", target_bir_lowering=False, debug=False, num_devices=8
    )

    x_d = nc.dram_tensor("x", [C, T], f16, kind="ExternalInput").ap()
    base_d = nc.dram_tensor("base", [C, T], f16, kind="ExternalInput").ap()
    top_d = nc.dram_tensor("top", [C, S], f16, kind="ExternalInput").ap()
    combt_d = nc.dram_tensor("combt", [S, C], f16, kind="ExternalInput").ap()
    w_d = nc.dram_tensor("w", [C, C], f16, kind="ExternalInput").ap()
    b_d = nc.dram_tensor("bvec", [P, KC], f32, kind="ExternalInput").ap()
    attn_d = nc.dram_tensor("attn_o", [T, S], f16, kind="ExternalOutput").ap()
    ctx_d = nc.dram_tensor("ctx_o", [C, T], f16, kind="ExternalOutput").ap()

    Exp = mybir.ActivationFunctionType.Exp
    X = mybir.AxisListType.X

    with tile.TileContext(nc) as tc, ExitStack() as ctx:
        res = ctx.enter_context(tc.tile_pool(name="resident", bufs=1))
        psum = ctx.enter_context(tc.tile_pool(name="psum", bufs=3, space="PSUM"))
        psum_ctx = ctx.enter_context(
            tc.tile_pool(name="psum_ctx", bufs=2, space="PSUM")
        )
        stats = ctx.enter_context(tc.tile_pool(name="stats", bufs=NT + 4))
        work = ctx.enter_context(tc.tile_pool(name="work", bufs=2))
        ctxo = ctx.enter_context(tc.tile_pool(name="ctxo", bufs=4))
        atp = ctx.enter_context(tc.tile_pool(name="attnT", bufs=1))

        # ---- resident loads ----
        top_t = res.tile([P, KC, S], f16, tag="top")
        nc.sync.dma_start(top_t[:], top_d.rearrange("(k p) s -> p k s", p=P))
        comb_t = res.tile([P, KS, C], f16, tag="combT")
        nc.sync.dma_start(comb_t[:], combt_d.rearrange("(k p) c -> p k c", p=P))
        target_t = res.tile([P, KC, T], f16, tag="target")

        # ---- preatt: target = W2.T @ x + b2 + base2 (scales pre-folded) ----
        with tc.tile_pool(name="pre", bufs=1) as pre, tc.tile_pool(
            name="t1", bufs=3
        ) as t1p:
            w_t = pre.tile([P, KC, C], f16, tag="w")
            nc.sync.dma_start(w_t[:], w_d.rearrange("(k p) o -> p k o", p=P))
            x_t = pre.tile([P, KC, T], f16, tag="x")
            nc.sync.dma_start(x_t[:], x_d.rearrange("(k p) t -> p k t", p=P))
            base_t = pre.tile([P, KC, T], f16, tag="base")
            nc.sync.dma_start(
                base_t[:], base_d.rearrange("(k p) t -> p k t", p=P)
            )
            b_t = pre.tile([P, KC], f32, tag="bvec")
            nc.sync.dma_start(b_t[:], b_d)

            for tci in range(NTC):
                tsl = slice(tci * NB, (tci + 1) * NB)
                for m in range(KC):
                    pp = psum.tile([P, NB], f32, tag="sc")
                    for kc in range(KC):
                        nc.tensor.matmul(
                            pp[:],
                            w_t[:, kc, m * P : (m + 1) * P],
                            x_t[:, kc, tsl],
                            start=(kc == 0),
                            stop=(kc == KC - 1),
                        )
                    t1 = t1p.tile([P, NB], f16, tag="t1")
                    nc.scalar.add(t1[:], pp[:], b_t[:, m : m + 1])
                    nc.vector.tensor_add(
                        target_t[:, m, tsl], t1[:], base_t[:, m, tsl]
                    )

        # ---- scores + softmax for all t-tiles; transposes per half ----
        attnT = [
            atp.tile([P, KS, TH], f16, tag=f"attnT{h}", name=f"attnT{h}")
            for h in range(2)
        ]
        attn_writes: list = []
        for i in range(NT):
            t0 = i * P
            ph = [
                psum.tile([P, 2 * NB], f32, tag="sc", name=f"sc{i}_{h2}")
                for h2 in range(2)
            ]
            # kc-outer so 4 consecutive matmuls share the stationary operand
            for kc in range(KC):
                lhsT = target_t[:, kc, t0 : t0 + P]
                for schunk in range(4):
                    ssl = slice(schunk * NB, (schunk + 1) * NB)
                    nc.tensor.matmul(
                        ph[schunk // 2][:, (schunk % 2) * NB : (schunk % 2 + 1) * NB],
                        lhsT,
                        top_t[:, kc, ssl],
                        start=(kc == 0),
                        stop=(kc == KC - 1),
                    )
            # softmax straight from PSUM (no staging copy)
            mx2 = stats.tile([P, 2], f32, tag="mx2")
            for h2 in range(2):
                nc.vector.reduce_max(mx2[:, h2 : h2 + 1], ph[h2][:], axis=X)
            negmax = stats.tile([P, 1], f32, tag="negmax")
            nc.vector.reduce_max(negmax[:], mx2[:], axis=X, negate=True)
            expt = work.tile([P, S], f16, tag="exp")
            rowsum2 = stats.tile([P, 2], f32, tag="rowsum2")
            for h2 in range(2):
                nc.scalar.activation(
                    expt[:, h2 * 2 * NB : (h2 + 1) * 2 * NB],
                    ph[h2][:],
                    Exp,
                    bias=negmax[:],
                    accum_out=rowsum2[:, h2 : h2 + 1],
                )
            rowsum = stats.tile([P, 1], f32, tag="rowsum")
            nc.vector.reduce_sum(rowsum[:], rowsum2[:], axis=X)
            rsum = stats.tile([P, 1], f32, tag="rsum")
            nc.vector.reciprocal(rsum[:], rowsum[:])
            attn = work.tile([P, S], f16, tag="attn")
            nc.vector.tensor_scalar_mul(attn[:], expt[:], rsum[:])
            attn_writes.append(
                nc.sync.dma_start(attn_d[t0 : t0 + P, :], attn[:])
            )
            if i % HT == HT - 1:
                h = i // HT
                deps = attn_writes[h * HT : (h + 1) * HT]
                for k in range(KS):
                    tr = nc.sync.dma_start_transpose(
                        attnT[h][:, k, :],
                        attn_d[h * TH : (h + 1) * TH, k * P : (k + 1) * P],
                    )
                    for w_inst in deps:
                        add_dep_helper(tr.ins, w_inst.ins, reason="attn dram RAW")

        # ---- ctx = combine @ attn^T, per half ----
        for h in range(2):
            for m in range(KC):
                pc = [
                    psum_ctx.tile([P, NB], f32, tag="ctx", name=f"ctx{h}_{m}_{t2}")
                    for t2 in range(2)
                ]
                # k-outer so both tc2 matmuls share the stationary operand
                for k in range(KS):
                    lhsT = comb_t[:, k, m * P : (m + 1) * P]
                    for tc2 in range(2):
                        nc.tensor.matmul(
                            pc[tc2][:],
                            lhsT,
                            attnT[h][:, k, tc2 * NB : (tc2 + 1) * NB],
                            start=(k == 0),
                            stop=(k == KS - 1),
                        )
                for tc2 in range(2):
                    co = ctxo.tile([P, NB], f16, tag="ctxo")
                    nc.scalar.copy(co[:], pc[tc2][:])
                    nc.sync.dma_start(
                        ctx_d[
                            m * P : (m + 1) * P,
                            h * TH + tc2 * NB : h * TH + (tc2 + 1) * NB,
                        ],
                        co[:],
                    )

    nc.compile()
    return nc


def _get_nc():
    key = (_C, _T, _S)
    if key not in _cache:
        _cache[key] = _build(*key)
    return _cache[key]


def _prep_in_maps(base_target_emb, input_from_dec, encoder_out_top,
                  encoder_out_combine, W, b):
    f16 = np.float16
    base = np.asarray(base_target_emb, dtype=np.float32)
    x = np.asarray(input_from_dec, dtype=np.float32)
    top = np.asarray(encoder_out_top, dtype=np.float32)
    comb = np.asarray(encoder_out_combine, dtype=np.float32)
    W = np.asarray(W, dtype=np.float32)
    b = np.asarray(b, dtype=np.float32)

    w2 = np.ascontiguousarray((W.T * _SW).astype(f16))          # [c_in, c_out]
    b2 = np.ascontiguousarray(
        (b * _SW).astype(np.float32).reshape(_C // _P, _P).T
    )                                                            # [128, KC]
    base2 = (base[..., 0] * _SW).astype(f16)                     # [B, C, T]
    x16 = x[..., 0].astype(f16)                                  # [B, C, T]
    top16 = top.astype(f16)                                      # [B, C, S]
    combt16 = comb.astype(f16).transpose(0, 2, 1)                # [B, S, C]

    in_maps = []
    for bi in range(base2.shape[0]):
        in_maps.append(
            {
                "x": np.ascontiguousarray(x16[bi]),
                "base": np.ascontiguousarray(base2[bi]),
                "top": np.ascontiguousarray(top16[bi]),
                "combt": np.ascontiguousarray(combt16[bi]),
                "w": w2,
                "bvec": b2,
            }
        )
    return in_maps


def kernel(base_target_emb, input_from_dec, encoder_out_top,
           encoder_out_combine, W, b):
    from concourse.bass_utils import run_bass_kernel_spmd

    nc = _get_nc()
    in_maps = _prep_in_maps(
        base_target_emb, input_from_dec, encoder_out_top,
        encoder_out_combine, W, b,
    )
    res = run_bass_kernel_spmd(nc, in_maps, core_ids=list(range(_B)))
    outs = res.results
    attn = np.stack(
        [outs[i]["attn_o"].astype(np.float32) for i in range(_B)]
    )                                                            # [B, T, S]
    ctx = np.stack(
        [outs[i]["ctx_o"].astype(np.float32) for i in range(_B)]
    )[..., None]                                                 # [B, C, T, 1]
    return ctx, attn


# revision 11
# speedup vs baseline: 1.0112x; 1.0112x over previous
"""Trainium2 Bass kernel for ConvMultiStepAttention.

Math (per batch element b):
    preatt = W @ x + b                      # [C,T], x = input_from_dec[b,:,:,0]
    target = (base + preatt) * sqrt(0.5)    # [C,T]
    scores = target.T @ top                 # [T,S]
    attn   = softmax(scores, axis=1)        # [T,S]   (output 2)
    ctx    = attn @ combine.T               # [T,C] -> stored as [C,T] (output 1)

Sharding: pure data parallel, one batch element per NeuronCore (B=8 = n_cores).

Precision: all matmuls run in fp16 (1 cycle/row on PE vs 4 for fp32) with fp32
PSUM accumulation; softmax stats (max/sum) in fp32.  sqrt(0.5) is folded into
W, b and base on the host.  Measured end-to-end absmax error vs the fp32
reference is ~1.1e-2 of output scale.

attn^T (needed as the ctx-matmul moving operand with the contraction dim on
partitions) is produced by reading the already-written attn fp16 DRAM output
back through the DMA xbar transpose in [1024, 128] blocks.  The xbar dispatch
cost is ~1.2us per *instruction* regardless of size, so few big jobs beat many
128x128 ones.  combine^T is pre-transposed on the host.
"""

import numpy as np

_B, _C, _T, _S = 8, 512, 2048, 2048
_SW = np.float32(0.5**0.5)
_P = 128
_NB = 512  # one PSUM bank in fp32 elements; also max fp32 matmul free dim

_cache: dict = {}


def _build(C: int, T: int, S: int):
    """Build + compile the single-core SPMD program. Returns the Bass object."""
    from contextlib import ExitStack

    import concourse.bacc as bacc
    import concourse.tile as tile
    from concourse import mybir
    from concourse.tile_rust import add_dep_helper

    f16 = mybir.dt.float16
    f32 = mybir.dt.float32
    P, NB = _P, _NB
    KC = C // P  # channel k-subtiles            (4)
    KS = S // P  # s k-subtiles for ctx matmul   (16)
    NT = T // P  # t row-tiles                   (16)
    NTC = T // NB  # t chunks of 512             (4)
    TH = T // 2  # t-half size                   (1024)
    HT = NT // 2  # t-tiles per half             (8)

    nc = bacc.Bacc(
        "TRN2", target_bir_lowering=False, debug=False, num_devices=8
    )

    x_d = nc.dram_tensor("x", [C, T], f16, kind="ExternalInput").ap()
    base_d = nc.dram_tensor("base", [C, T], f16, kind="ExternalInput").ap()
    top_d = nc.dram_tensor("top", [C, S], f16, kind="ExternalInput").ap()
    combt_d = nc.dram_tensor("combt", [S, C], f16, kind="ExternalInput").ap()
    w_d = nc.dram_tensor("w", [C, C], f16, kind="ExternalInput").ap()
    b_d = nc.dram_tensor("bvec", [P, KC], f32, kind="ExternalInput").ap()
    attn_d = nc.dram_tensor("attn_o", [T, S], f16, kind="ExternalOutput").ap()
    ctx_d = nc.dram_tensor("ctx_o", [C, T], f16, kind="ExternalOutput").ap()

    Exp = mybir.ActivationFunctionType.Exp
    X = mybir.AxisListType.X

    with tile.TileContext(nc) as tc, ExitStack() as ctx:
        res = ctx.enter_context(tc.tile_pool(name="resident", bufs=1))
        psum = ctx.enter_context(tc.tile_pool(name="psum", bufs=4, space="PSUM"))
        stats = ctx.enter_context(tc.tile_pool(name="stats", bufs=NT + 4))
        work = ctx.enter_context(tc.tile_pool(name="work", bufs=2))
        ctxo = ctx.enter_context(tc.tile_pool(name="ctxo", bufs=4))
        atp = ctx.enter_context(tc.tile_pool(name="attnT", bufs=1))

        # ---- resident loads (split per k-subtile so compute starts early) ----
        top_t = res.tile([P, KC, S], f16, tag="top")
        top_r = top_d.rearrange("(k p) s -> p k s", p=P)
        for k in range(KC):
            nc.sync.dma_start(top_t[:, k, :], top_r[:, k, :])
        comb_t = res.tile([P, KS, C], f16, tag="combT")
        comb_r = combt_d.rearrange("(k p) c -> p k c", p=P)
        for k in range(0, KS, 4):
            nc.sync.dma_start(comb_t[:, k : k + 4, :], comb_r[:, k : k + 4, :])
        target_t = res.tile([P, KC, T], f16, tag="target")

        # ---- preatt: target = W2.T @ x + b2 + base2 (scales pre-folded) ----
        with tc.tile_pool(name="pre", bufs=1) as pre, tc.tile_pool(
            name="t1", bufs=3
        ) as t1p:
            w_t = pre.tile([P, KC, C], f16, tag="w")
            w_r = w_d.rearrange("(k p) o -> p k o", p=P)
            x_t = pre.tile([P, KC, T], f16, tag="x")
            x_r = x_d.rearrange("(k p) t -> p k t", p=P)
            base_t = pre.tile([P, KC, T], f16, tag="base")
            base_r = base_d.rearrange("(k p) t -> p k t", p=P)
            for k in range(KC):
                nc.sync.dma_start(w_t[:, k, :], w_r[:, k, :])
                nc.sync.dma_start(x_t[:, k, :], x_r[:, k, :])
                nc.sync.dma_start(base_t[:, k, :], base_r[:, k, :])
            b_t = pre.tile([P, KC], f32, tag="bvec")
            nc.sync.dma_start(b_t[:], b_d)

            for tci in range(NTC):
                tsl = slice(tci * NB, (tci + 1) * NB)
                for m in range(KC):
                    pp = psum.tile([P, NB], f32, tag="sc")
                    for kc in range(KC):
                        nc.tensor.matmul(
                            pp[:],
                            w_t[:, kc, m * P : (m + 1) * P],
                            x_t[:, kc, tsl],
                            start=(kc == 0),
                            stop=(kc == KC - 1),
                        )
                    t1 = t1p.tile([P, NB], f16, tag="t1")
                    nc.scalar.add(t1[:], pp[:], b_t[:, m : m + 1])
                    nc.vector.tensor_add(
                        target_t[:, m, tsl], t1[:], base_t[:, m, tsl]
                    )

        # ---- scores + softmax for all t-tiles; transposes per half ----
        attnT = [
            atp.tile([P, KS, TH], f16, tag=f"attnT{h}", name=f"attnT{h}")
            for h in range(2)
        ]
        attn_writes: list = []
        for i in range(NT):
            t0 = i * P
            ph = [
                psum.tile([P, 2 * NB], f32, tag="sc", name=f"sc{i}_{h2}")
                for h2 in range(2)
            ]
            # kc-outer so 4 consecutive matmuls share the stationary operand
            for kc in range(KC):
                lhsT = target_t[:, kc, t0 : t0 + P]
                for schunk in range(4):
                    ssl = slice(schunk * NB, (schunk + 1) * NB)
                    nc.tensor.matmul(
                        ph[schunk // 2][:, (schunk % 2) * NB : (schunk % 2 + 1) * NB],
                        lhsT,
                        top_t[:, kc, ssl],
                        start=(kc == 0),
                        stop=(kc == KC - 1),
                    )
            # softmax straight from PSUM (no staging copy)
            mx2 = stats.tile([P, 2], f32, tag="mx2")
            for h2 in range(2):
                nc.vector.reduce_max(mx2[:, h2 : h2 + 1], ph[h2][:], axis=X)
            negmax = stats.tile([P, 1], f32, tag="negmax")
            nc.vector.reduce_max(negmax[:], mx2[:], axis=X, negate=True)
            expt = work.tile([P, S], f16, tag="exp")
            rowsum2 = stats.tile([P, 2], f32, tag="rowsum2")
            for h2 in range(2):
                nc.scalar.activation(
                    expt[:, h2 * 2 * NB : (h2 + 1) * 2 * NB],
                    ph[h2][:],
                    Exp,
                    bias=negmax[:],
                    accum_out=rowsum2[:, h2 : h2 + 1],
                )
            rowsum = stats.tile([P, 1], f32, tag="rowsum")
            nc.vector.reduce_sum(rowsum[:], rowsum2[:], axis=X)
            rsum = stats.tile([P, 1], f32, tag="rsum")
            nc.vector.reciprocal(rsum[:], rowsum[:])
            attn = work.tile([P, S], f16, tag="attn")
            nc.vector.tensor_scalar_mul(attn[:], expt[:], rsum[:])
            attn_writes.append(
                nc.sync.dma_start(attn_d[t0 : t0 + P, :], attn[:])
            )
            if i % HT == HT - 1:
                h = i // HT
                deps = attn_writes[h * HT : (h + 1) * HT]
                for k in range(KS):
                    tr = nc.sync.dma_start_transpose(
                        attnT[h][:, k, :],
                        attn_d[h * TH : (h + 1) * TH, k * P : (k + 1) * P],
                    )
                    for w_inst in deps:
                        add_dep_helper(tr.ins, w_inst.ins, reason="attn dram RAW")

        # ---- ctx = combine @ attn^T, per half ----
        for h in range(2):
            for m in range(KC):
                pc = [
                    psum.tile([P, NB], f32, tag="sc", name=f"ctx{h}_{m}_{t2}")
                    for t2 in range(2)
                ]
                # k-outer so both tc2 matmuls share the stationary operand
                for k in range(KS):
                    lhsT = comb_t[:, k, m * P : (m + 1) * P]
                    for tc2 in range(2):
                        nc.tensor.matmul(
                            pc[tc2][:],
                            lhsT,
                            attnT[h][:, k, tc2 * NB : (tc2 + 1) * NB],
                            start=(k == 0),
                            stop=(k == KS - 1),
                        )
                for tc2 in range(2):
                    co = ctxo.tile([P, NB], f16, tag="ctxo")
                    nc.scalar.copy(co[:], pc[tc2][:])
                    nc.sync.dma_start(
                        ctx_d[
                            m * P : (m + 1) * P,
                            h * TH + tc2 * NB : h * TH + (tc2 + 1) * NB,
                        ],
                        co[:],
                    )

    nc.compile()
    return nc


def _get_nc():
    key = (_C, _T, _S)
    if key not in _cache:
        _cache[key] = _build(*key)
    return _cache[key]


def _prep_in_maps(base_target_emb, input_from_dec, encoder_out_top,
                  encoder_out_combine, W, b):
    f16 = np.float16
    base = np.asarray(base_target_emb, dtype=np.float32)
    x = np.asarray(input_from_dec, dtype=np.float32)
    top = np.asarray(encoder_out_top, dtype=np.float32)
    comb = np.asarray(encoder_out_combine, dtype=np.float32)
    W = np.asarray(W, dtype=np.float32)
    b = np.asarray(b, dtype=np.float32)

    w2 = np.ascontiguousarray((W.T * _SW).astype(f16))          # [c_in, c_out]
    b2 = np.ascontiguousarray(
        (b * _SW).astype(np.float32).reshape(_C // _P, _P).T
    )                                                            # [128, KC]
    base2 = (base[..., 0] * _SW).astype(f16)                     # [B, C, T]
    x16 = x[..., 0].astype(f16)                                  # [B, C, T]
    top16 = top.astype(f16)                                      # [B, C, S]
    combt16 = comb.astype(f16).transpose(0, 2, 1)                # [B, S, C]

    in_maps = []
    for bi in range(base2.shape[0]):
        in_maps.append(
            {
                "x": np.ascontiguousarray(x16[bi]),
                "base": np.ascontiguousarray(base2[bi]),
                "top": np.ascontiguousarray(top16[bi]),
                "combt": np.ascontiguousarray(combt16[bi]),
                "w": w2,
                "bvec": b2,
            }
        )
    return in_maps


def kernel(base_target_emb, input_from_dec, encoder_out_top,
           encoder_out_combine, W, b):
    from concourse.bass_utils import run_bass_kernel_spmd

    nc = _get_nc()
    in_maps = _prep_in_maps(
        base_target_emb, input_from_dec, encoder_out_top,
        encoder_out_combine, W, b,
    )
    res = run_bass_kernel_spmd(nc, in_maps, core_ids=list(range(_B)))
    outs = res.results
    attn = np.stack(
        [outs[i]["attn_o"].astype(np.float32) for i in range(_B)]
    )                                                            # [B, T, S]
    ctx = np.stack(
        [outs[i]["ctx_o"].astype(np.float32) for i in range(_B)]
    )[..., None]                                                 # [B, C, T, 1]
    return ctx, attn


# revision 17
# speedup vs baseline: 1.1078x; 1.0956x over previous
"""Trainium2 Bass kernel for ConvMultiStepAttention.

Math (per batch element b):
    preatt = W @ x + b                      # [C,T], x = input_from_dec[b,:,:,0]
    target = (base + preatt) * sqrt(0.5)    # [C,T]
    scores = target.T @ top                 # [T,S]
    attn   = softmax(scores, axis=1)        # [T,S]   (output 2)
    ctx    = attn @ combine.T               # [T,C] -> stored as [C,T] (output 1)

Sharding: pure data parallel, one batch element per NeuronCore (B=8 = n_cores).

Precision: all matmuls run in fp16 (1 cycle/row on PE vs 4 for fp32) with fp32
PSUM accumulation; softmax stats (max/sum) in fp32.  sqrt(0.5) is folded into
W, b and base on the host.  Measured end-to-end absmax error vs the fp32
reference is ~1.1e-2 of output scale.

attn^T (needed as the ctx-matmul moving operand with the contraction dim on
partitions) is produced by reading the already-written attn fp16 DRAM output
back through the DMA xbar transpose in [1024, 128] blocks.  The xbar dispatch
cost is ~1.2us per *instruction* regardless of size, so few big jobs beat many
128x128 ones.  combine^T is pre-transposed on the host.
"""

import numpy as np

_B, _C, _T, _S = 8, 512, 2048, 2048
_SW = np.float32(0.5**0.5)
_P = 128
_NB = 512  # one PSUM bank in fp32 elements; also max fp32 matmul free dim

_cache: dict = {}


def _build(C: int, T: int, S: int):
    """Build + compile the single-core SPMD program. Returns the Bass object."""
    from contextlib import ExitStack

    import concourse.bacc as bacc
    import concourse.tile as tile
    from concourse import mybir
    from concourse.tile_rust import add_dep_helper

    f16 = mybir.dt.float16
    f32 = mybir.dt.float32
    P, NB = _P, _NB
    KC = C // P  # channel k-subtiles            (4)
    KS = S // P  # s k-subtiles for ctx matmul   (16)
    NT = T // P  # t row-tiles                   (16)
    NTC = T // NB  # t chunks of 512             (4)
    TH = T // 2  # t-half size                   (1024)
    HT = NT // 2  # t-tiles per half             (8)

    nc = bacc.Bacc(
        "TRN2", target_bir_lowering=False, debug=False, num_devices=8
    )

    x_d = nc.dram_tensor("x", [C, T], f16, kind="ExternalInput").ap()
    base_d = nc.dram_tensor("base", [C, T], f16, kind="ExternalInput").ap()
    top_d = nc.dram_tensor("top", [C, S], f16, kind="ExternalInput").ap()
    combt_d = nc.dram_tensor("combt", [S, C], f16, kind="ExternalInput").ap()
    w_d = nc.dram_tensor("w", [C, C], f16, kind="ExternalInput").ap()
    b_d = nc.dram_tensor("bvec", [P, KC], f32, kind="ExternalInput").ap()
    attn_d = nc.dram_tensor("attn_o", [T, S], f16, kind="ExternalOutput").ap()
    ctx_d = nc.dram_tensor("ctx_o", [C, T], f16, kind="ExternalOutput").ap()

    Exp = mybir.ActivationFunctionType.Exp
    X = mybir.AxisListType.X

    with tile.TileContext(nc) as tc, ExitStack() as ctx:
        res = ctx.enter_context(tc.tile_pool(name="resident", bufs=1))
        psum = ctx.enter_context(tc.tile_pool(name="psum", bufs=4, space="PSUM"))
        stats = ctx.enter_context(tc.tile_pool(name="stats", bufs=NT + 4))
        work = ctx.enter_context(tc.tile_pool(name="work", bufs=2))
        ctxo = ctx.enter_context(tc.tile_pool(name="ctxo", bufs=4))
        atp = ctx.enter_context(tc.tile_pool(name="attnT", bufs=1))

        # ---- resident tiles; loads ordered by when compute needs them ----
        top_t = res.tile([P, KC, S], f16, tag="top")
        top_r = top_d.rearrange("(k p) s -> p k s", p=P)
        comb_t = res.tile([P, KS, C], f16, tag="combT")
        comb_r = combt_d.rearrange("(k p) c -> p k c", p=P)
        target_t = res.tile([P, KC, T], f16, tag="target")

        # ---- preatt: target = W2.T @ x + b2 + base2 (scales pre-folded) ----
        with tc.tile_pool(name="pre", bufs=1) as pre, tc.tile_pool(
            name="t1", bufs=3
        ) as t1p:
            w_t = pre.tile([P, KC, C], f16, tag="w")
            w_r = w_d.rearrange("(k p) o -> p k o", p=P)
            x_t = pre.tile([P, KC, T], f16, tag="x")
            x_r = x_d.rearrange("(k p) t -> p k t", p=P)
            base_t = pre.tile([P, KC, T], f16, tag="base")
            base_r = base_d.rearrange("(k p) t -> p k t", p=P)
            for k in range(KC):
                nc.sync.dma_start(w_t[:, k, :], w_r[:, k, :])
                nc.sync.dma_start(x_t[:, k, :], x_r[:, k, :])
                nc.sync.dma_start(base_t[:, k, :], base_r[:, k, :])
            for k in range(KC):
                nc.scalar.dma_start(top_t[:, k, :], top_r[:, k, :])
            for k in range(0, KS, 8):
                nc.scalar.dma_start(
                    comb_t[:, k : k + 8, :], comb_r[:, k : k + 8, :]
                )
            b_t = pre.tile([P, KC], f32, tag="bvec")
            nc.sync.dma_start(b_t[:], b_d)

            for tci in range(NTC):
                tsl = slice(tci * NB, (tci + 1) * NB)
                for m in range(KC):
                    pp = psum.tile([P, NB], f32, tag="sc")
                    for kc in range(KC):
                        nc.tensor.matmul(
                            pp[:],
                            w_t[:, kc, m * P : (m + 1) * P],
                            x_t[:, kc, tsl],
                            start=(kc == 0),
                            stop=(kc == KC - 1),
                        )
                    t1 = t1p.tile([P, NB], f16, tag="t1")
                    nc.scalar.add(t1[:], pp[:], b_t[:, m : m + 1])
                    nc.vector.tensor_add(
                        target_t[:, m, tsl], t1[:], base_t[:, m, tsl]
                    )

        # ---- scores + softmax for all t-tiles; transposes per half ----
        # one tile per (half, k) so ctx matmuls depend only on their own
        # transpose, not the whole half
        attnT = [
            [
                atp.tile([P, TH], f16, tag=f"attnT{h}_{k}", name=f"attnT{h}_{k}")
                for k in range(KS)
            ]
            for h in range(2)
        ]
        attn_writes: list = []
        for i in range(NT):
            t0 = i * P
            ph = [
                psum.tile([P, 2 * NB], f32, tag="sc", name=f"sc{i}_{h2}")
                for h2 in range(2)
            ]
            # kc-outer so 4 consecutive matmuls share the stationary operand
            for kc in range(KC):
                lhsT = target_t[:, kc, t0 : t0 + P]
                for schunk in range(4):
                    ssl = slice(schunk * NB, (schunk + 1) * NB)
                    nc.tensor.matmul(
                        ph[schunk // 2][:, (schunk % 2) * NB : (schunk % 2 + 1) * NB],
                        lhsT,
                        top_t[:, kc, ssl],
                        start=(kc == 0),
                        stop=(kc == KC - 1),
                    )
            # softmax straight from PSUM (no staging copy)
            mx2 = stats.tile([P, 2], f32, tag="mx2")
            for h2 in range(2):
                nc.vector.reduce_max(mx2[:, h2 : h2 + 1], ph[h2][:], axis=X)
            negmax = stats.tile([P, 1], f32, tag="negmax")
            nc.vector.reduce_max(negmax[:], mx2[:], axis=X, negate=True)
            expt = work.tile([P, S], f16, tag="exp")
            rowsum2 = stats.tile([P, 2], f32, tag="rowsum2")
            for h2 in range(2):
                nc.scalar.activation(
                    expt[:, h2 * 2 * NB : (h2 + 1) * 2 * NB],
                    ph[h2][:],
                    Exp,
                    bias=negmax[:],
                    accum_out=rowsum2[:, h2 : h2 + 1],
                )
            rowsum = stats.tile([P, 1], f32, tag="rowsum")
            nc.vector.reduce_sum(rowsum[:], rowsum2[:], axis=X)
            rsum = stats.tile([P, 1], f32, tag="rsum")
            nc.vector.reciprocal(rsum[:], rowsum[:])
            attn = work.tile([P, S], f16, tag="attn")
            nc.vector.tensor_scalar_mul(attn[:], expt[:], rsum[:])
            attn_writes.append(
                nc.sync.dma_start(attn_d[t0 : t0 + P, :], attn[:])
            )
            if i % HT == HT - 1:
                h = i // HT
                deps = attn_writes[h * HT : (h + 1) * HT]
                for k in range(KS):
                    eng = nc.sync
                    tr = eng.dma_start_transpose(
                        attnT[h][k][:],
                        attn_d[h * TH : (h + 1) * TH, k * P : (k + 1) * P],
                    )
                    for w_inst in deps:
                        add_dep_helper(tr.ins, w_inst.ins, reason="attn dram RAW")

        # ---- ctx = combine @ attn^T, per half ----
        for h in range(2):
            for m in range(KC):
                pc = [
                    psum.tile([P, NB], f32, tag="sc", name=f"ctx{h}_{m}_{t2}")
                    for t2 in range(2)
                ]
                # k-outer so both tc2 matmuls share the stationary operand
                for k in range(KS):
                    lhsT = comb_t[:, k, m * P : (m + 1) * P]
                    for tc2 in range(2):
                        nc.tensor.matmul(
                            pc[tc2][:],
                            lhsT,
                            attnT[h][k][:, tc2 * NB : (tc2 + 1) * NB],
                            start=(k == 0),
                            stop=(k == KS - 1),
                        )
                co = ctxo.tile([P, 2 * NB], f16, tag="ctxo")
                for tc2 in range(2):
                    nc.scalar.copy(co[:, tc2 * NB : (tc2 + 1) * NB], pc[tc2][:])
                nc.sync.dma_start(
                    ctx_d[m * P : (m + 1) * P, h * TH : (h + 1) * TH], co[:]
                )

    nc.compile()
    return nc


def _get_nc():
    key = (_C, _T, _S)
    if key not in _cache:
        _cache[key] = _build(*key)
    return _cache[key]


def _prep_in_maps(base_target_emb, input_from_dec, encoder_out_top,
                  encoder_out_combine, W, b):
    f16 = np.float16
    base = np.asarray(base_target_emb, dtype=np.float32)
    x = np.asarray(input_from_dec, dtype=np.float32)
    top = np.asarray(encoder_out_top, dtype=np.float32)
    comb = np.asarray(encoder_out_combine, dtype=np.float32)
    W = np.asarray(W, dtype=np.float32)
    b = np.asarray(b, dtype=np.float32)

    w2 = np.ascontiguousarray((W.T * _SW).astype(f16))          # [c_in, c_out]
    b2 = np.ascontiguousarray(
        (b * _SW).astype(np.float32).reshape(_C // _P, _P).T
    )                                                            # [128, KC]
    base2 = (base[..., 0] * _SW).astype(f16)                     # [B, C, T]
    x16 = x[..., 0].astype(f16)                                  # [B, C, T]
    top16 = top.astype(f16)                                      # [B, C, S]
    combt16 = comb.astype(f16).transpose(0, 2, 1)                # [B, S, C]

    in_maps = []
    for bi in range(base2.shape[0]):
        in_maps.append(
            {
                "x": np.ascontiguousarray(x16[bi]),
                "base": np.ascontiguousarray(base2[bi]),
                "top": np.ascontiguousarray(top16[bi]),
                "combt": np.ascontiguousarray(combt16[bi]),
                "w": w2,
                "bvec": b2,
            }
        )
    return in_maps


def kernel(base_target_emb, input_from_dec, encoder_out_top,
           encoder_out_combine, W, b):
    from concourse.bass_utils import run_bass_kernel_spmd

    nc = _get_nc()
    in_maps = _prep_in_maps(
        base_target_emb, input_from_dec, encoder_out_top,
        encoder_out_combine, W, b,
    )
    res = run_bass_kernel_spmd(nc, in_maps, core_ids=list(range(_B)))
    outs = res.results
    attn = np.stack(
        [outs[i]["attn_o"].astype(np.float32) for i in range(_B)]
    )                                                            # [B, T, S]
    ctx = np.stack(
        [outs[i]["ctx_o"].astype(np.float32) for i in range(_B)]
    )[..., None]                                                 # [B, C, T, 1]
    return ctx, attn
